# revision 9
# baseline (speedup 1.0000x reference)
"""Chamfer distance kernel for Trainium2 (8 NeuronCores, SPMD).

Problem: f, f_ ~ [4, 8192, 128] fp32.
  dis[b,n,m] = ||f[b,n] - f_[b,m]||^2
  out = mean_b( mean_n min_m dis + mean_m min_n dis )

Sharding: 8 cores = (batch b in 0..3) x (n-half h in 0..1).
Core (b,h) computes the [4096, 8192] block of the NEGATED distance matrix
  nd[n,m] = 2<f_n, f_m'> - |f_n|^2 - |f_m'|^2  (= -dis)
on-chip. Matmuls are bf16 (inputs rounded on host), PSUM accumulates fp32.
The two squared-norm terms are applied by a mix of engines chosen to
balance their throughput (PE streams ~0.83 ns/col here):
  - "aug" row-blocks: an extra K=2 matmul accumulates both norm terms
    into PSUM (costs a second PE stream of the strip).
  - "noaug" row-blocks: -|f_n|^2 is applied as a per-partition bias in the
    ACT PSUM->SBUF copy (free), and -|f_m'|^2 by a GPSIMD tensor-add
    (in-place on the fp16 SBUF tile; GPSIMD is otherwise idle).
Reductions per core (all on fp16 SBUF tiles, DVE 2x mode):
  - rowmax (-> min_m dis for each of 4096 rows): TT-max fold tree
    (8192 -> 2048 -> 1024 -> 512) + one small 1x reduce tail.
  - colmax (-> partial min_n dis): running elementwise TT-max into a
    [128, 8192] fp16 accumulator; the remaining max over the 128
    partitions and over the two n-halves is done on host.
Host does input marshalling (transpose, 2x scale, squared norms, bf16
rounding) and the tiny final gather/means.
"""

import sys

for _p in ("/opt/trn_rl_repo",):
    if _p not in sys.path:
        sys.path.insert(0, _p)

import numpy as np
import ml_dtypes

import concourse.bass as bass
import concourse.mybir as mybir
import concourse.tile as tile
from concourse import bacc
from concourse.bass_utils import run_bass_kernel_spmd

F32 = mybir.dt.float32
F16 = mybir.dt.float16
BF16 = mybir.dt.bfloat16

B, N, C = 4, 8192, 128
NCORES = 8
NH = N // 2              # 4096 rows per core
NT = NH // 128           # 32 n-tile row-blocks per core
SW = 2048                # psum strip width (4 banks)
NS = N // SW             # 4 strips per row-block
ACC_INIT = -60000.0      # < min possible nd value; safe in fp16


def _is_aug(i):
    # ~62% of row-blocks use the aug matmul; the rest go via ACT bias +
    # a DVE add. Interleaved to smooth engine load.
    return (i % 8) < 5


_PROGRAM = None


def _build_program():
    nc = bacc.Bacc("TRN2", target_bir_lowering=False, debug=False)

    d_at = nc.dram_tensor("at", [C, NH], BF16, kind="ExternalInput")
    d_bt2 = nc.dram_tensor("bt2", [C, N], BF16, kind="ExternalInput")
    d_augw = nc.dram_tensor("augw", [2, NH], BF16, kind="ExternalInput")
    d_augm = nc.dram_tensor("augm", [2, N], BF16, kind="ExternalInput")
    d_nasq = nc.dram_tensor("nasq", [128, NT], F32, kind="ExternalInput")
    d_nbsq = nc.dram_tensor("nbsq", [1, N], F16, kind="ExternalInput")
    d_rowmax = nc.dram_tensor("rowmax", [128, NT], F32, kind="ExternalOutput")
    d_colacc = nc.dram_tensor("colacc", [128, N], F16, kind="ExternalOutput")

    with tile.TileContext(nc) as tc:
        with (
            tc.tile_pool(name="singles", bufs=1) as singles,
            tc.tile_pool(name="gpool", bufs=10) as gpool,
            tc.tile_pool(name="fpool", bufs=4) as fpool,
            tc.tile_pool(name="hpool", bufs=4) as hpool,
            tc.tile_pool(name="psum", bufs=2, space="PSUM") as psum_pool,
        ):
            at_sb = singles.tile([C, NH], BF16)
            bt2_sb = singles.tile([C, N], BF16)
            augw_sb = singles.tile([2, NH], BF16)
            augm_sb = singles.tile([2, N], BF16)
            nasq_sb = singles.tile([128, NT], F32)
            nbsq_rep = singles.tile([128, N], F16)
            for q in range(4):
                nc.sync.dma_start(
                    out=at_sb[:, q * (NH // 4):(q + 1) * (NH // 4)],
                    in_=d_at[:, q * (NH // 4):(q + 1) * (NH // 4)],
                )
                nc.sync.dma_start(
                    out=bt2_sb[:, q * (N // 4):(q + 1) * (N // 4)],
                    in_=d_bt2[:, q * (N // 4):(q + 1) * (N // 4)],
                )
            nc.sync.dma_start(out=augw_sb, in_=d_augw[:])
            nc.sync.dma_start(out=augm_sb, in_=d_augm[:])
            nc.sync.dma_start(out=nasq_sb, in_=d_nasq[:])
            # broadcast -|f_m'|^2 to all 128 partitions
            nbsq_bcast = bass.AP(
                tensor=d_nbsq[:].tensor,
                offset=d_nbsq[:].offset,
                ap=[[0, 128], [1, N]],
            )
            nc.sync.dma_start(out=nbsq_rep, in_=nbsq_bcast)

            acc = singles.tile([128, N], F16)
            nc.vector.memset(acc, ACC_INIT)
            rowmax_sb = singles.tile([128, NT], F32)

            for i in range(NT):
                aug = _is_aug(i)
                w_sl = slice(i * 128, (i + 1) * 128)
                g_tiles = []
                for jj in range(NS):
                    ps = psum_pool.tile([128, SW], F32)
                    base = jj * SW
                    for k in range(SW // 512):
                        c_sl = slice(base + k * 512, base + (k + 1) * 512)
                        p_sl = slice(k * 512, (k + 1) * 512)
                        nc.tensor.matmul(
                            out=ps[:, p_sl], lhsT=at_sb[:, w_sl],
                            rhs=bt2_sb[:, c_sl], start=True, stop=not aug,
                        )
                    if aug:
                        for k in range(SW // 512):
                            c_sl = slice(base + k * 512, base + (k + 1) * 512)
                            p_sl = slice(k * 512, (k + 1) * 512)
                            nc.tensor.matmul(
                                out=ps[:, p_sl], lhsT=augw_sb[:, w_sl],
                                rhs=augm_sb[:, c_sl], start=False, stop=True,
                            )
                    g = gpool.tile([128, SW], F16, tag="g")
                    if aug:
                        nc.scalar.copy(out=g[:], in_=ps[:])
                    else:
                        # g = psum + (-|f_n|^2)  (per-partition bias)
                        nc.scalar.activation(
                            out=g[:], in_=ps[:],
                            func=mybir.ActivationFunctionType.Identity,
                            bias=nasq_sb[:, i:i + 1], scale=1.0,
                        )
                        # g += -|f_m'|^2  (DVE fp16 2x, in place)
                        nc.vector.tensor_tensor(
                            out=g[:], in0=g[:],
                            in1=nbsq_rep[:, base:base + SW],
                            op=mybir.AluOpType.add,
                        )
                    g_tiles.append(g)
                    # colmax running fold (DVE, fp16 2x)
                    a_sl = slice(base, base + SW)
                    nc.vector.tensor_tensor(
                        out=acc[:, a_sl], in0=acc[:, a_sl], in1=g[:],
                        op=mybir.AluOpType.max,
                    )
                # rowmax fold tree for this row-block (DVE fp16 2x) + 1x tail
                f0 = fpool.tile([128, SW], F16, tag="f")
                nc.vector.tensor_tensor(out=f0[:], in0=g_tiles[0][:], in1=g_tiles[1][:], op=mybir.AluOpType.max)
                f1 = fpool.tile([128, SW], F16, tag="f")
                nc.vector.tensor_tensor(out=f1[:], in0=g_tiles[2][:], in1=g_tiles[3][:], op=mybir.AluOpType.max)
                f2 = fpool.tile([128, SW], F16, tag="f")
                nc.vector.tensor_tensor(out=f2[:], in0=f0[:], in1=f1[:], op=mybir.AluOpType.max)
                h1 = hpool.tile([128, SW // 2], F16, tag="h1")
                nc.vector.tensor_tensor(out=h1[:], in0=f2[:, :SW // 2], in1=f2[:, SW // 2:], op=mybir.AluOpType.max)
                h2 = hpool.tile([128, SW // 4], F16, tag="h2")
                nc.vector.tensor_tensor(out=h2[:], in0=h1[:, :SW // 4], in1=h1[:, SW // 4:], op=mybir.AluOpType.max)
                nc.vector.tensor_reduce(
                    out=rowmax_sb[:, i:i + 1], in_=h2[:],
                    axis=mybir.AxisListType.X, op=mybir.AluOpType.max,
                )

            nc.sync.dma_start(out=d_rowmax[:], in_=rowmax_sb[:])
            for q in range(2):
                nc.sync.dma_start(
                    out=d_colacc[:, q * (N // 2):(q + 1) * (N // 2)],
                    in_=acc[:, q * (N // 2):(q + 1) * (N // 2)],
                )

    nc.compile()
    return nc


def _get_program():
    global _PROGRAM
    if _PROGRAM is None:
        _PROGRAM = _build_program()
    return _PROGRAM


def _prep_core_inputs(f, f_):
    """Per-core host marshalling: transpose + scale + squared norms."""
    in_maps = []
    for c in range(NCORES):
        b, h = divmod(c, 2)
        A = f[b, h * NH:(h + 1) * NH]        # [4096, 128]
        Bm = f_[b]                           # [8192, 128]
        at = np.ascontiguousarray(A.T.astype(ml_dtypes.bfloat16))
        bt2 = np.ascontiguousarray((2.0 * Bm.T).astype(ml_dtypes.bfloat16))
        asq = (A.astype(np.float64) ** 2).sum(-1).astype(np.float32)
        bsq = (Bm.astype(np.float64) ** 2).sum(-1).astype(np.float32)
        augw = np.ascontiguousarray(np.stack([np.ones(NH, np.float32), -asq]).astype(ml_dtypes.bfloat16))
        augm = np.ascontiguousarray(np.stack([-bsq, np.ones(N, np.float32)]).astype(ml_dtypes.bfloat16))
        # noaug-path constants (full fp32/fp16 precision)
        nasq = np.ascontiguousarray((-asq).reshape(NT, 128).T.astype(np.float32))
        nbsq = np.ascontiguousarray((-bsq).astype(np.float16).reshape(1, N))
        in_maps.append({
            "at": at, "bt2": bt2, "augw": augw, "augm": augm,
            "nasq": nasq, "nbsq": nbsq,
        })
    return in_maps


def _finalize(results):
    """Host-side gather: tiny final reductions + means (fp64)."""
    d_sum = 0.0
    for b in range(B):
        r0 = results[2 * b]
        r1 = results[2 * b + 1]
        f2f_0 = -r0["rowmax"].astype(np.float64).T.reshape(-1)   # [4096]
        f2f_1 = -r1["rowmax"].astype(np.float64).T.reshape(-1)
        mean_f2f = (f2f_0.sum() + f2f_1.sum()) / N
        cm = np.maximum(
            r0["colacc"].astype(np.float32).max(axis=0),
            r1["colacc"].astype(np.float32).max(axis=0),
        ).astype(np.float64)
        mean_f_2f = (-cm).mean()
        d_sum += mean_f2f + mean_f_2f
    return np.float32(d_sum / B)


def kernel(f, f_):
    f = np.asarray(f, dtype=np.float32)
    f_ = np.asarray(f_, dtype=np.float32)
    nc = _get_program()
    in_maps = _prep_core_inputs(f, f_)
    res = run_bass_kernel_spmd(nc, in_maps, list(range(NCORES)))
    return _finalize(res.results)


if __name__ == "__main__":
    rng = np.random.default_rng(0)
    f = rng.standard_normal((B, N, C), dtype=np.float32)
    f_ = rng.standard_normal((B, N, C), dtype=np.float32)
    out = kernel(f, f_)
    print("kernel out:", out)


# revision 10
# speedup vs baseline: 1.3133x; 1.3133x over previous
"""Chamfer distance kernel for Trainium2 (8 NeuronCores, SPMD).

Problem: f, f_ ~ [4, 8192, 128] fp32.
  dis[b,n,m] = ||f[b,n] - f_[b,m]||^2
  out = mean_b( mean_n min_m dis + mean_m min_n dis )

Sharding: 8 cores = (batch b in 0..3) x (n-half h in 0..1).
Core (b,h) computes the [4096, 8192] block of the NEGATED distance matrix
  nd[n,m] = 2<f_n, f_m'> - |f_n|^2 - |f_m'|^2  (= -dis)
on-chip. Matmuls are bf16 (inputs rounded on host), PSUM accumulates fp32.
The two squared-norm terms are applied by a mix of engines chosen to
balance their throughput (PE streams ~0.83 ns/col here):
  - "aug" row-blocks: an extra K=2 matmul accumulates both norm terms
    into PSUM (costs a second PE stream of the strip).
  - "noaug" row-blocks: -|f_n|^2 is applied as a per-partition bias in the
    ACT PSUM->SBUF copy (free), and -|f_m'|^2 by a GPSIMD tensor-add
    (in-place on the fp16 SBUF tile; GPSIMD is otherwise idle).
Reductions per core (all on fp16 SBUF tiles, DVE 2x mode):
  - rowmax (-> min_m dis for each of 4096 rows): TT-max fold tree
    (8192 -> 2048 -> 1024 -> 512) + one small 1x reduce tail.
  - colmax (-> partial min_n dis): running elementwise TT-max into a
    [128, 8192] fp16 accumulator; the remaining max over the 128
    partitions and over the two n-halves is done on host.
Host does input marshalling (transpose, 2x scale, squared norms, bf16
rounding) and the tiny final gather/means.
"""

import sys

for _p in ("/opt/trn_rl_repo",):
    if _p not in sys.path:
        sys.path.insert(0, _p)

import numpy as np
import ml_dtypes

import concourse.bass as bass
import concourse.mybir as mybir
import concourse.tile as tile
from concourse import bacc
from concourse.bass_utils import run_bass_kernel_spmd

F32 = mybir.dt.float32
F16 = mybir.dt.float16
BF16 = mybir.dt.bfloat16

B, N, C = 4, 8192, 128
NCORES = 8
NH = N // 2              # 4096 rows per core
NT = NH // 128           # 32 n-tile row-blocks per core
SW = 2048                # psum strip width (4 banks)
NS = N // SW             # 4 strips per row-block
ACC_INIT = -60000.0      # < min possible nd value; safe in fp16


def _is_aug(i):
    # ~2/3 of row-blocks use the aug matmul; the rest go via ACT bias +
    # a DVE add. Interleaved (A A N) to smooth engine load; tuned on HW.
    return (i % 3) < 2


_PROGRAM = None


def _build_program():
    nc = bacc.Bacc("TRN2", target_bir_lowering=False, debug=False)

    d_at = nc.dram_tensor("at", [C, NH], BF16, kind="ExternalInput")
    d_bt2 = nc.dram_tensor("bt2", [C, N], BF16, kind="ExternalInput")
    d_augw = nc.dram_tensor("augw", [2, NH], BF16, kind="ExternalInput")
    d_augm = nc.dram_tensor("augm", [2, N], BF16, kind="ExternalInput")
    d_nasq = nc.dram_tensor("nasq", [128, NT], F32, kind="ExternalInput")
    d_nbsq = nc.dram_tensor("nbsq", [1, N], F16, kind="ExternalInput")
    d_rowmax = nc.dram_tensor("rowmax", [128, NT], F32, kind="ExternalOutput")
    d_colacc = nc.dram_tensor("colacc", [128, N], F16, kind="ExternalOutput")

    with tile.TileContext(nc) as tc:
        with (
            tc.tile_pool(name="singles", bufs=1) as singles,
            tc.tile_pool(name="gpool", bufs=10) as gpool,
            tc.tile_pool(name="fpool", bufs=4) as fpool,
            tc.tile_pool(name="hpool", bufs=4) as hpool,
            tc.tile_pool(name="psum", bufs=2, space="PSUM") as psum_pool,
        ):
            at_sb = singles.tile([C, NH], BF16)
            bt2_sb = singles.tile([C, N], BF16)
            augw_sb = singles.tile([2, NH], BF16)
            augm_sb = singles.tile([2, N], BF16)
            nasq_sb = singles.tile([128, NT], F32)
            nbsq_rep = singles.tile([128, N], F16)
            # small tensors first (block 0 needs augw/augm), then the big
            # ones in fine chunks so the first strips can start early
            nc.sync.dma_start(out=augw_sb, in_=d_augw[:])
            nc.sync.dma_start(out=augm_sb, in_=d_augm[:])
            nc.sync.dma_start(out=nasq_sb, in_=d_nasq[:])
            nc.sync.dma_start(out=at_sb[:, 0:512], in_=d_at[:, 0:512])
            for q in range(8):
                nc.sync.dma_start(
                    out=bt2_sb[:, q * (N // 8):(q + 1) * (N // 8)],
                    in_=d_bt2[:, q * (N // 8):(q + 1) * (N // 8)],
                )
            for q in range(1, 8):
                nc.sync.dma_start(
                    out=at_sb[:, q * 512:(q + 1) * 512],
                    in_=d_at[:, q * 512:(q + 1) * 512],
                )
            # broadcast -|f_m'|^2 to all 128 partitions
            nbsq_bcast = bass.AP(
                tensor=d_nbsq[:].tensor,
                offset=d_nbsq[:].offset,
                ap=[[0, 128], [1, N]],
            )
            nc.sync.dma_start(out=nbsq_rep, in_=nbsq_bcast)

            acc = singles.tile([128, N], F16)
            nc.vector.memset(acc, ACC_INIT)
            rowmax_sb = singles.tile([128, NT], F32)

            for i in range(NT):
                aug = _is_aug(i)
                w_sl = slice(i * 128, (i + 1) * 128)
                g_tiles = []
                for jj in range(NS):
                    ps = psum_pool.tile([128, SW], F32)
                    base = jj * SW
                    for k in range(SW // 512):
                        c_sl = slice(base + k * 512, base + (k + 1) * 512)
                        p_sl = slice(k * 512, (k + 1) * 512)
                        nc.tensor.matmul(
                            out=ps[:, p_sl], lhsT=at_sb[:, w_sl],
                            rhs=bt2_sb[:, c_sl], start=True, stop=not aug,
                        )
                    if aug:
                        for k in range(SW // 512):
                            c_sl = slice(base + k * 512, base + (k + 1) * 512)
                            p_sl = slice(k * 512, (k + 1) * 512)
                            nc.tensor.matmul(
                                out=ps[:, p_sl], lhsT=augw_sb[:, w_sl],
                                rhs=augm_sb[:, c_sl], start=False, stop=True,
                            )
                    g = gpool.tile([128, SW], F16, tag="g")
                    if aug:
                        nc.scalar.copy(out=g[:], in_=ps[:])
                    else:
                        # g = psum + (-|f_n|^2)  (per-partition bias)
                        nc.scalar.activation(
                            out=g[:], in_=ps[:],
                            func=mybir.ActivationFunctionType.Identity,
                            bias=nasq_sb[:, i:i + 1], scale=1.0,
                        )
                        # g += -|f_m'|^2  (DVE fp16 2x, in place)
                        nc.vector.tensor_tensor(
                            out=g[:], in0=g[:],
                            in1=nbsq_rep[:, base:base + SW],
                            op=mybir.AluOpType.add,
                        )
                    g_tiles.append(g)
                    # colmax running fold (DVE, fp16 2x)
                    a_sl = slice(base, base + SW)
                    nc.vector.tensor_tensor(
                        out=acc[:, a_sl], in0=acc[:, a_sl], in1=g[:],
                        op=mybir.AluOpType.max,
                    )
                # rowmax fold tree for this row-block (DVE fp16 2x) + 1x tail
                f0 = fpool.tile([128, SW], F16, tag="f")
                nc.vector.tensor_tensor(out=f0[:], in0=g_tiles[0][:], in1=g_tiles[1][:], op=mybir.AluOpType.max)
                f1 = fpool.tile([128, SW], F16, tag="f")
                nc.vector.tensor_tensor(out=f1[:], in0=g_tiles[2][:], in1=g_tiles[3][:], op=mybir.AluOpType.max)
                f2 = fpool.tile([128, SW], F16, tag="f")
                nc.vector.tensor_tensor(out=f2[:], in0=f0[:], in1=f1[:], op=mybir.AluOpType.max)
                h1 = hpool.tile([128, SW // 2], F16, tag="h1")
                nc.vector.tensor_tensor(out=h1[:], in0=f2[:, :SW // 2], in1=f2[:, SW // 2:], op=mybir.AluOpType.max)
                h2 = hpool.tile([128, SW // 4], F16, tag="h2")
                nc.vector.tensor_tensor(out=h2[:], in0=h1[:, :SW // 4], in1=h1[:, SW // 4:], op=mybir.AluOpType.max)
                nc.vector.tensor_reduce(
                    out=rowmax_sb[:, i:i + 1], in_=h2[:],
                    axis=mybir.AxisListType.X, op=mybir.AluOpType.max,
                )

            nc.sync.dma_start(out=d_rowmax[:], in_=rowmax_sb[:])
            for q in range(2):
                nc.sync.dma_start(
                    out=d_colacc[:, q * (N // 2):(q + 1) * (N // 2)],
                    in_=acc[:, q * (N // 2):(q + 1) * (N // 2)],
                )

    nc.compile()
    return nc


def _get_program():
    global _PROGRAM
    if _PROGRAM is None:
        _PROGRAM = _build_program()
    return _PROGRAM


def _prep_core_inputs(f, f_):
    """Per-core host marshalling: transpose + scale + squared norms."""
    in_maps = []
    for c in range(NCORES):
        b, h = divmod(c, 2)
        A = f[b, h * NH:(h + 1) * NH]        # [4096, 128]
        Bm = f_[b]                           # [8192, 128]
        at = np.ascontiguousarray(A.T.astype(ml_dtypes.bfloat16))
        bt2 = np.ascontiguousarray((2.0 * Bm.T).astype(ml_dtypes.bfloat16))
        asq = (A.astype(np.float64) ** 2).sum(-1).astype(np.float32)
        bsq = (Bm.astype(np.float64) ** 2).sum(-1).astype(np.float32)
        augw = np.ascontiguousarray(np.stack([np.ones(NH, np.float32), -asq]).astype(ml_dtypes.bfloat16))
        augm = np.ascontiguousarray(np.stack([-bsq, np.ones(N, np.float32)]).astype(ml_dtypes.bfloat16))
        # noaug-path constants (full fp32/fp16 precision)
        nasq = np.ascontiguousarray((-asq).reshape(NT, 128).T.astype(np.float32))
        nbsq = np.ascontiguousarray((-bsq).astype(np.float16).reshape(1, N))
        in_maps.append({
            "at": at, "bt2": bt2, "augw": augw, "augm": augm,
            "nasq": nasq, "nbsq": nbsq,
        })
    return in_maps


def _finalize(results):
    """Host-side gather: tiny final reductions + means (fp64)."""
    d_sum = 0.0
    for b in range(B):
        r0 = results[2 * b]
        r1 = results[2 * b + 1]
        f2f_0 = -r0["rowmax"].astype(np.float64).T.reshape(-1)   # [4096]
        f2f_1 = -r1["rowmax"].astype(np.float64).T.reshape(-1)
        mean_f2f = (f2f_0.sum() + f2f_1.sum()) / N
        cm = np.maximum(
            r0["colacc"].astype(np.float32).max(axis=0),
            r1["colacc"].astype(np.float32).max(axis=0),
        ).astype(np.float64)
        mean_f_2f = (-cm).mean()
        d_sum += mean_f2f + mean_f_2f
    return np.float32(d_sum / B)


def kernel(f, f_):
    f = np.asarray(f, dtype=np.float32)
    f_ = np.asarray(f_, dtype=np.float32)
    nc = _get_program()
    in_maps = _prep_core_inputs(f, f_)
    res = run_bass_kernel_spmd(nc, in_maps, list(range(NCORES)))
    return _finalize(res.results)


if __name__ == "__main__":
    rng = np.random.default_rng(0)
    f = rng.standard_normal((B, N, C), dtype=np.float32)
    f_ = rng.standard_normal((B, N, C), dtype=np.float32)
    out = kernel(f, f_)
    print("kernel out:", out)


# revision 11
# speedup vs baseline: 1.3333x; 1.0153x over previous
"""Chamfer distance kernel for Trainium2 (8 NeuronCores, SPMD).

Problem: f, f_ ~ [4, 8192, 128] fp32.
  dis[b,n,m] = ||f[b,n] - f_[b,m]||^2
  out = mean_b( mean_n min_m dis + mean_m min_n dis )

Sharding: 8 cores = (batch b in 0..3) x (n-half h in 0..1).
Core (b,h) computes the [4096, 8192] block of the NEGATED distance matrix
  nd[n,m] = 2<f_n, f_m'> - |f_n|^2 - |f_m'|^2  (= -dis)
on-chip. Matmuls are bf16 (inputs rounded on host), PSUM accumulates fp32.
The two squared-norm terms are applied by a mix of engines chosen to
balance their throughput (PE streams ~0.83 ns/col here):
  - "aug" row-blocks: an extra K=2 matmul accumulates both norm terms
    into PSUM (costs a second PE stream of the strip).
  - "noaug" row-blocks: -|f_n|^2 is applied as a per-partition bias in the
    ACT PSUM->SBUF copy (free), and -|f_m'|^2 by a GPSIMD tensor-add
    (in-place on the fp16 SBUF tile; GPSIMD is otherwise idle).
Reductions per core (all on fp16 SBUF tiles, DVE 2x mode):
  - rowmax (-> min_m dis for each of 4096 rows): TT-max fold tree over a
    per-block [128, 8192] fp16 tile (8192 -> 4096 -> 2048 -> 1024 -> 512)
    + one small 1x reduce tail.
  - colmax (-> partial min_n dis): running elementwise TT-max into a
    [128, 8192] fp16 accumulator; the remaining max over the 128
    partitions and over the two n-halves is done on host.
Host does input marshalling (transpose, 2x scale, squared norms, bf16
rounding) and the tiny final gather/means.
"""

import sys

for _p in ("/opt/trn_rl_repo",):
    if _p not in sys.path:
        sys.path.insert(0, _p)

import numpy as np
import ml_dtypes

import concourse.bass as bass
import concourse.mybir as mybir
import concourse.tile as tile
from concourse import bacc
from concourse.bass_utils import run_bass_kernel_spmd

F32 = mybir.dt.float32
F16 = mybir.dt.float16
BF16 = mybir.dt.bfloat16

B, N, C = 4, 8192, 128
NCORES = 8
NH = N // 2              # 4096 rows per core
NT = NH // 128           # 32 n-tile row-blocks per core
SW = 2048                # psum strip width (4 banks)
NS = N // SW             # 4 strips per row-block
ACC_INIT = -60000.0      # < min possible nd value; safe in fp16


def _is_aug(i):
    # ~2/3 of row-blocks use the aug matmul; the rest go via ACT bias +
    # a DVE add. Interleaved (A A N) to smooth engine load; tuned on HW.
    return (i % 3) < 2


_PROGRAM = None


def _build_program():
    nc = bacc.Bacc("TRN2", target_bir_lowering=False, debug=False)

    d_at = nc.dram_tensor("at", [C, NH], BF16, kind="ExternalInput")
    d_bt2 = nc.dram_tensor("bt2", [C, N], BF16, kind="ExternalInput")
    d_augw = nc.dram_tensor("augw", [2, NH], BF16, kind="ExternalInput")
    d_augm = nc.dram_tensor("augm", [2, N], BF16, kind="ExternalInput")
    d_nasq = nc.dram_tensor("nasq", [128, NT], F32, kind="ExternalInput")
    d_nbsq = nc.dram_tensor("nbsq", [1, N], F16, kind="ExternalInput")
    d_rowmax = nc.dram_tensor("rowmax", [128, NT], F32, kind="ExternalOutput")
    d_colacc = nc.dram_tensor("colacc", [128, N], F16, kind="ExternalOutput")

    with tile.TileContext(nc) as tc:
        with (
            tc.tile_pool(name="singles", bufs=1) as singles,
            tc.tile_pool(name="gpool", bufs=3) as gpool,
            tc.tile_pool(name="fpool", bufs=3) as fpool,
            tc.tile_pool(name="hpool", bufs=4) as hpool,
            tc.tile_pool(name="psum", bufs=2, space="PSUM") as psum_pool,
        ):
            at_sb = singles.tile([C, NH], BF16)
            bt2_sb = singles.tile([C, N], BF16)
            augw_sb = singles.tile([2, NH], BF16)
            augm_sb = singles.tile([2, N], BF16)
            nasq_sb = singles.tile([128, NT], F32)
            nbsq_rep = singles.tile([128, N], F16)
            # small tensors first (block 0 needs augw/augm), then the big
            # ones in fine chunks so the first strips can start early
            nc.sync.dma_start(out=augw_sb, in_=d_augw[:])
            nc.sync.dma_start(out=augm_sb, in_=d_augm[:])
            nc.sync.dma_start(out=nasq_sb, in_=d_nasq[:])
            nc.sync.dma_start(out=at_sb[:, 0:512], in_=d_at[:, 0:512])
            for q in range(8):
                nc.sync.dma_start(
                    out=bt2_sb[:, q * (N // 8):(q + 1) * (N // 8)],
                    in_=d_bt2[:, q * (N // 8):(q + 1) * (N // 8)],
                )
            for q in range(1, 8):
                nc.sync.dma_start(
                    out=at_sb[:, q * 512:(q + 1) * 512],
                    in_=d_at[:, q * 512:(q + 1) * 512],
                )
            # broadcast -|f_m'|^2 to all 128 partitions
            nbsq_bcast = bass.AP(
                tensor=d_nbsq[:].tensor,
                offset=d_nbsq[:].offset,
                ap=[[0, 128], [1, N]],
            )
            nc.sync.dma_start(out=nbsq_rep, in_=nbsq_bcast)

            acc = singles.tile([128, N], F16)
            nc.vector.memset(acc, ACC_INIT)
            rowmax_sb = singles.tile([128, NT], F32)

            for i in range(NT):
                aug = _is_aug(i)
                w_sl = slice(i * 128, (i + 1) * 128)
                g = gpool.tile([128, N], F16, tag="g")
                for jj in range(NS):
                    ps = psum_pool.tile([128, SW], F32)
                    base = jj * SW
                    for k in range(SW // 512):
                        c_sl = slice(base + k * 512, base + (k + 1) * 512)
                        p_sl = slice(k * 512, (k + 1) * 512)
                        nc.tensor.matmul(
                            out=ps[:, p_sl], lhsT=at_sb[:, w_sl],
                            rhs=bt2_sb[:, c_sl], start=True, stop=not aug,
                        )
                    if aug:
                        for k in range(SW // 512):
                            c_sl = slice(base + k * 512, base + (k + 1) * 512)
                            p_sl = slice(k * 512, (k + 1) * 512)
                            nc.tensor.matmul(
                                out=ps[:, p_sl], lhsT=augw_sb[:, w_sl],
                                rhs=augm_sb[:, c_sl], start=False, stop=True,
                            )
                    gs = g[:, base:base + SW]
                    if aug:
                        nc.scalar.copy(out=gs, in_=ps[:])
                    else:
                        # g = psum + (-|f_n|^2)  (per-partition bias)
                        nc.scalar.activation(
                            out=gs, in_=ps[:],
                            func=mybir.ActivationFunctionType.Identity,
                            bias=nasq_sb[:, i:i + 1], scale=1.0,
                        )
                        # g += -|f_m'|^2  (DVE fp16 2x, in place)
                        nc.vector.tensor_tensor(
                            out=gs, in0=gs,
                            in1=nbsq_rep[:, base:base + SW],
                            op=mybir.AluOpType.add,
                        )
                # colmax running fold (DVE, fp16 2x), two 4096-wide ops
                for q in range(2):
                    a_sl = slice(q * (N // 2), (q + 1) * (N // 2))
                    nc.vector.tensor_tensor(
                        out=acc[:, a_sl], in0=acc[:, a_sl], in1=g[:, a_sl],
                        op=mybir.AluOpType.max,
                    )
                # rowmax fold tree for this row-block (DVE fp16 2x) + 1x tail
                f1 = fpool.tile([128, N // 2], F16, tag="f1")
                nc.vector.tensor_tensor(out=f1[:], in0=g[:, :N // 2], in1=g[:, N // 2:], op=mybir.AluOpType.max)
                f2 = fpool.tile([128, SW], F16, tag="f2")
                nc.vector.tensor_tensor(out=f2[:], in0=f1[:, :SW], in1=f1[:, SW:], op=mybir.AluOpType.max)
                h1 = hpool.tile([128, SW // 2], F16, tag="h1")
                nc.vector.tensor_tensor(out=h1[:], in0=f2[:, :SW // 2], in1=f2[:, SW // 2:], op=mybir.AluOpType.max)
                h2 = hpool.tile([128, SW // 4], F16, tag="h2")
                nc.vector.tensor_tensor(out=h2[:], in0=h1[:, :SW // 4], in1=h1[:, SW // 4:], op=mybir.AluOpType.max)
                nc.vector.tensor_reduce(
                    out=rowmax_sb[:, i:i + 1], in_=h2[:],
                    axis=mybir.AxisListType.X, op=mybir.AluOpType.max,
                )

            nc.sync.dma_start(out=d_rowmax[:], in_=rowmax_sb[:])
            for q in range(2):
                nc.sync.dma_start(
                    out=d_colacc[:, q * (N // 2):(q + 1) * (N // 2)],
                    in_=acc[:, q * (N // 2):(q + 1) * (N // 2)],
                )

    nc.compile()
    return nc


def _get_program():
    global _PROGRAM
    if _PROGRAM is None:
        _PROGRAM = _build_program()
    return _PROGRAM


def _prep_core_inputs(f, f_):
    """Per-core host marshalling: transpose + scale + squared norms."""
    in_maps = []
    for c in range(NCORES):
        b, h = divmod(c, 2)
        A = f[b, h * NH:(h + 1) * NH]        # [4096, 128]
        Bm = f_[b]                           # [8192, 128]
        at = np.ascontiguousarray(A.T.astype(ml_dtypes.bfloat16))
        bt2 = np.ascontiguousarray((2.0 * Bm.T).astype(ml_dtypes.bfloat16))
        asq = (A.astype(np.float64) ** 2).sum(-1).astype(np.float32)
        bsq = (Bm.astype(np.float64) ** 2).sum(-1).astype(np.float32)
        augw = np.ascontiguousarray(np.stack([np.ones(NH, np.float32), -asq]).astype(ml_dtypes.bfloat16))
        augm = np.ascontiguousarray(np.stack([-bsq, np.ones(N, np.float32)]).astype(ml_dtypes.bfloat16))
        # noaug-path constants (full fp32/fp16 precision)
        nasq = np.ascontiguousarray((-asq).reshape(NT, 128).T.astype(np.float32))
        nbsq = np.ascontiguousarray((-bsq).astype(np.float16).reshape(1, N))
        in_maps.append({
            "at": at, "bt2": bt2, "augw": augw, "augm": augm,
            "nasq": nasq, "nbsq": nbsq,
        })
    return in_maps


def _finalize(results):
    """Host-side gather: tiny final reductions + means (fp64)."""
    d_sum = 0.0
    for b in range(B):
        r0 = results[2 * b]
        r1 = results[2 * b + 1]
        f2f_0 = -r0["rowmax"].astype(np.float64).T.reshape(-1)   # [4096]
        f2f_1 = -r1["rowmax"].astype(np.float64).T.reshape(-1)
        mean_f2f = (f2f_0.sum() + f2f_1.sum()) / N
        cm = np.maximum(
            r0["colacc"].astype(np.float32).max(axis=0),
            r1["colacc"].astype(np.float32).max(axis=0),
        ).astype(np.float64)
        mean_f_2f = (-cm).mean()
        d_sum += mean_f2f + mean_f_2f
    return np.float32(d_sum / B)


def kernel(f, f_):
    f = np.asarray(f, dtype=np.float32)
    f_ = np.asarray(f_, dtype=np.float32)
    nc = _get_program()
    in_maps = _prep_core_inputs(f, f_)
    res = run_bass_kernel_spmd(nc, in_maps, list(range(NCORES)))
    return _finalize(res.results)


if __name__ == "__main__":
    rng = np.random.default_rng(0)
    f = rng.standard_normal((B, N, C), dtype=np.float32)
    f_ = rng.standard_normal((B, N, C), dtype=np.float32)
    out = kernel(f, f_)
    print("kernel out:", out)


# revision 12
# speedup vs baseline: 1.3386x; 1.0039x over previous
"""Chamfer distance kernel for Trainium2 (8 NeuronCores, SPMD).

Problem: f, f_ ~ [4, 8192, 128] fp32.
  dis[b,n,m] = ||f[b,n] - f_[b,m]||^2
  out = mean_b( mean_n min_m dis + mean_m min_n dis )

Sharding: 8 cores = (batch b in 0..3) x (n-half h in 0..1).
Core (b,h) computes the [4096, 8192] block of the NEGATED distance matrix
  nd[n,m] = 2<f_n, f_m'> - |f_n|^2 - |f_m'|^2  (= -dis)
on-chip. Matmuls are bf16 (inputs rounded on host), PSUM accumulates fp32.
The two squared-norm terms are applied by a mix of engines chosen to
balance their throughput (PE streams ~0.83 ns/col here):
  - "aug" row-blocks: an extra K=2 matmul accumulates both norm terms
    into PSUM (costs a second PE stream of the strip).
  - "noaug" row-blocks: -|f_n|^2 is applied as a per-partition bias in the
    ACT PSUM->SBUF copy (free), and -|f_m'|^2 by a GPSIMD tensor-add
    (in-place on the fp16 SBUF tile; GPSIMD is otherwise idle).
Reductions per core (all on fp16 SBUF tiles, DVE 2x mode):
  - rowmax (-> min_m dis for each of 4096 rows): TT-max fold tree over a
    per-block [128, 8192] fp16 tile (8192 -> 4096 -> 2048 -> 1024 -> 512)
    + one small 1x reduce tail.
  - colmax (-> partial min_n dis): running elementwise TT-max into a
    [128, 8192] fp16 accumulator; the remaining max over the 128
    partitions and over the two n-halves is done on host.
Host does input marshalling (transpose, 2x scale, squared norms, bf16
rounding) and the tiny final gather/means.
"""

import sys

for _p in ("/opt/trn_rl_repo",):
    if _p not in sys.path:
        sys.path.insert(0, _p)

import numpy as np
import ml_dtypes

import concourse.bass as bass
import concourse.mybir as mybir
import concourse.tile as tile
from concourse import bacc
from concourse.bass_utils import run_bass_kernel_spmd

F32 = mybir.dt.float32
F16 = mybir.dt.float16
BF16 = mybir.dt.bfloat16

B, N, C = 4, 8192, 128
NCORES = 8
NH = N // 2              # 4096 rows per core
NT = NH // 128           # 32 n-tile row-blocks per core
SW = 2048                # psum strip width (4 banks)
NS = N // SW             # 4 strips per row-block


def _is_aug(i):
    # ~2/3 of row-blocks use the aug matmul; the rest go via ACT bias +
    # a DVE add. Interleaved (A A N) to smooth engine load; tuned on HW.
    return (i % 3) < 2


_PROGRAM = None


def _build_program():
    nc = bacc.Bacc("TRN2", target_bir_lowering=False, debug=False)

    d_at = nc.dram_tensor("at", [C, NH], BF16, kind="ExternalInput")
    d_bt2 = nc.dram_tensor("bt2", [C, N], BF16, kind="ExternalInput")
    d_augw = nc.dram_tensor("augw", [2, NH], BF16, kind="ExternalInput")
    d_augm = nc.dram_tensor("augm", [2, N], BF16, kind="ExternalInput")
    d_nasq = nc.dram_tensor("nasq", [128, NT], F32, kind="ExternalInput")
    d_nbsq = nc.dram_tensor("nbsq", [1, N], F16, kind="ExternalInput")
    d_rowmax = nc.dram_tensor("rowmax", [128, NT], F32, kind="ExternalOutput")
    d_colacc = nc.dram_tensor("colacc", [128, N], F16, kind="ExternalOutput")

    with tile.TileContext(nc) as tc:
        with (
            tc.tile_pool(name="singles", bufs=1) as singles,
            tc.tile_pool(name="gpool", bufs=3) as gpool,
            tc.tile_pool(name="fpool", bufs=3) as fpool,
            tc.tile_pool(name="hpool", bufs=4) as hpool,
            tc.tile_pool(name="psum", bufs=2, space="PSUM") as psum_pool,
        ):
            at_sb = singles.tile([C, NH], BF16)
            bt2_sb = singles.tile([C, N], BF16)
            augw_sb = singles.tile([2, NH], BF16)
            augm_sb = singles.tile([2, N], BF16)
            nasq_sb = singles.tile([128, NT], F32)
            nbsq_rep = singles.tile([128, N], F16)
            # small tensors first (block 0 needs augw/augm), then the big
            # ones in fine chunks so the first strips can start early
            nc.sync.dma_start(out=augw_sb, in_=d_augw[:])
            nc.sync.dma_start(out=augm_sb, in_=d_augm[:])
            nc.sync.dma_start(out=nasq_sb, in_=d_nasq[:])
            nc.sync.dma_start(out=at_sb[:, 0:512], in_=d_at[:, 0:512])
            for q in range(8):
                nc.sync.dma_start(
                    out=bt2_sb[:, q * (N // 8):(q + 1) * (N // 8)],
                    in_=d_bt2[:, q * (N // 8):(q + 1) * (N // 8)],
                )
            for q in range(1, 8):
                nc.sync.dma_start(
                    out=at_sb[:, q * 512:(q + 1) * 512],
                    in_=d_at[:, q * 512:(q + 1) * 512],
                )
            # broadcast -|f_m'|^2 to all 128 partitions
            nbsq_bcast = bass.AP(
                tensor=d_nbsq[:].tensor,
                offset=d_nbsq[:].offset,
                ap=[[0, 128], [1, N]],
            )
            nc.sync.dma_start(out=nbsq_rep, in_=nbsq_bcast)

            acc = singles.tile([128, N], F16)
            rowmax_sb = singles.tile([128, NT], F32)

            for i in range(NT):
                aug = _is_aug(i)
                w_sl = slice(i * 128, (i + 1) * 128)
                g = gpool.tile([128, N], F16, tag="g")
                for jj in range(NS):
                    ps = psum_pool.tile([128, SW], F32)
                    base = jj * SW
                    for k in range(SW // 512):
                        c_sl = slice(base + k * 512, base + (k + 1) * 512)
                        p_sl = slice(k * 512, (k + 1) * 512)
                        nc.tensor.matmul(
                            out=ps[:, p_sl], lhsT=at_sb[:, w_sl],
                            rhs=bt2_sb[:, c_sl], start=True, stop=not aug,
                        )
                    if aug:
                        for k in range(SW // 512):
                            c_sl = slice(base + k * 512, base + (k + 1) * 512)
                            p_sl = slice(k * 512, (k + 1) * 512)
                            nc.tensor.matmul(
                                out=ps[:, p_sl], lhsT=augw_sb[:, w_sl],
                                rhs=augm_sb[:, c_sl], start=False, stop=True,
                            )
                    gs = g[:, base:base + SW]
                    if aug:
                        nc.scalar.copy(out=gs, in_=ps[:])
                    else:
                        # g = psum + (-|f_n|^2)  (per-partition bias)
                        nc.scalar.activation(
                            out=gs, in_=ps[:],
                            func=mybir.ActivationFunctionType.Identity,
                            bias=nasq_sb[:, i:i + 1], scale=1.0,
                        )
                        # g += -|f_m'|^2  (DVE fp16 2x, in place)
                        nc.vector.tensor_tensor(
                            out=gs, in0=gs,
                            in1=nbsq_rep[:, base:base + SW],
                            op=mybir.AluOpType.add,
                        )
                # colmax running fold (DVE, fp16 2x), one 8192-wide op;
                # first block initializes acc with a 4x-mode copy instead
                if i == 0:
                    nc.vector.tensor_copy(acc[:], g[:])
                else:
                    nc.vector.tensor_tensor(
                        out=acc[:], in0=acc[:], in1=g[:],
                        op=mybir.AluOpType.max,
                    )
                # rowmax fold tree for this row-block (DVE fp16 2x) + 1x tail
                f1 = fpool.tile([128, N // 2], F16, tag="f1")
                nc.vector.tensor_tensor(out=f1[:], in0=g[:, :N // 2], in1=g[:, N // 2:], op=mybir.AluOpType.max)
                f2 = fpool.tile([128, SW], F16, tag="f2")
                nc.vector.tensor_tensor(out=f2[:], in0=f1[:, :SW], in1=f1[:, SW:], op=mybir.AluOpType.max)
                h1 = hpool.tile([128, SW // 2], F16, tag="h1")
                nc.vector.tensor_tensor(out=h1[:], in0=f2[:, :SW // 2], in1=f2[:, SW // 2:], op=mybir.AluOpType.max)
                h2 = hpool.tile([128, SW // 4], F16, tag="h2")
                nc.vector.tensor_tensor(out=h2[:], in0=h1[:, :SW // 4], in1=h1[:, SW // 4:], op=mybir.AluOpType.max)
                nc.vector.tensor_reduce(
                    out=rowmax_sb[:, i:i + 1], in_=h2[:],
                    axis=mybir.AxisListType.X, op=mybir.AluOpType.max,
                )

            nc.sync.dma_start(out=d_rowmax[:], in_=rowmax_sb[:])
            for q in range(2):
                nc.sync.dma_start(
                    out=d_colacc[:, q * (N // 2):(q + 1) * (N // 2)],
                    in_=acc[:, q * (N // 2):(q + 1) * (N // 2)],
                )

    nc.compile()
    return nc


def _get_program():
    global _PROGRAM
    if _PROGRAM is None:
        _PROGRAM = _build_program()
    return _PROGRAM


def _prep_core_inputs(f, f_):
    """Per-core host marshalling: transpose + scale + squared norms."""
    in_maps = []
    for c in range(NCORES):
        b, h = divmod(c, 2)
        A = f[b, h * NH:(h + 1) * NH]        # [4096, 128]
        Bm = f_[b]                           # [8192, 128]
        at = np.ascontiguousarray(A.T.astype(ml_dtypes.bfloat16))
        bt2 = np.ascontiguousarray((2.0 * Bm.T).astype(ml_dtypes.bfloat16))
        asq = (A.astype(np.float64) ** 2).sum(-1).astype(np.float32)
        bsq = (Bm.astype(np.float64) ** 2).sum(-1).astype(np.float32)
        augw = np.ascontiguousarray(np.stack([np.ones(NH, np.float32), -asq]).astype(ml_dtypes.bfloat16))
        augm = np.ascontiguousarray(np.stack([-bsq, np.ones(N, np.float32)]).astype(ml_dtypes.bfloat16))
        # noaug-path constants (full fp32/fp16 precision)
        nasq = np.ascontiguousarray((-asq).reshape(NT, 128).T.astype(np.float32))
        nbsq = np.ascontiguousarray((-bsq).astype(np.float16).reshape(1, N))
        in_maps.append({
            "at": at, "bt2": bt2, "augw": augw, "augm": augm,
            "nasq": nasq, "nbsq": nbsq,
        })
    return in_maps


def _finalize(results):
    """Host-side gather: tiny final reductions + means (fp64)."""
    d_sum = 0.0
    for b in range(B):
        r0 = results[2 * b]
        r1 = results[2 * b + 1]
        f2f_0 = -r0["rowmax"].astype(np.float64).T.reshape(-1)   # [4096]
        f2f_1 = -r1["rowmax"].astype(np.float64).T.reshape(-1)
        mean_f2f = (f2f_0.sum() + f2f_1.sum()) / N
        cm = np.maximum(
            r0["colacc"].astype(np.float32).max(axis=0),
            r1["colacc"].astype(np.float32).max(axis=0),
        ).astype(np.float64)
        mean_f_2f = (-cm).mean()
        d_sum += mean_f2f + mean_f_2f
    return np.float32(d_sum / B)


def kernel(f, f_):
    f = np.asarray(f, dtype=np.float32)
    f_ = np.asarray(f_, dtype=np.float32)
    nc = _get_program()
    in_maps = _prep_core_inputs(f, f_)
    res = run_bass_kernel_spmd(nc, in_maps, list(range(NCORES)))
    return _finalize(res.results)


if __name__ == "__main__":
    rng = np.random.default_rng(0)
    f = rng.standard_normal((B, N, C), dtype=np.float32)
    f_ = rng.standard_normal((B, N, C), dtype=np.float32)
    out = kernel(f, f_)
    print("kernel out:", out)


# revision 13
# speedup vs baseline: 1.3388x; 1.0002x over previous
"""Chamfer distance kernel for Trainium2 (8 NeuronCores, SPMD).

Problem: f, f_ ~ [4, 8192, 128] fp32.
  dis[b,n,m] = ||f[b,n] - f_[b,m]||^2
  out = mean_b( mean_n min_m dis + mean_m min_n dis )

Sharding: 8 cores = (batch b in 0..3) x (n-half h in 0..1).
Core (b,h) computes the [4096, 8192] block of the NEGATED distance matrix
  nd[n,m] = 2<f_n, f_m'> - |f_n|^2 - |f_m'|^2  (= -dis)
on-chip. Matmuls are bf16 (inputs rounded on host), PSUM accumulates fp32.
The two squared-norm terms are applied by a mix of engines chosen to
balance their throughput (PE streams ~0.83 ns/col here):
  - "aug" row-blocks: an extra K=2 matmul accumulates both norm terms
    into PSUM (costs a second PE stream of the strip).
  - "noaug" row-blocks: -|f_n|^2 is applied as a per-partition bias in the
    ACT PSUM->SBUF copy (free), and -|f_m'|^2 by a DVE tensor-add
    (in-place on the fp16 SBUF tile, 2x mode).
Reductions per core (all on fp16 SBUF tiles, DVE 2x mode):
  - rowmax (-> min_m dis for each of 4096 rows): TT-max fold tree over a
    per-block [128, 8192] fp16 tile (8192 -> 4096 -> 2048 -> 1024 -> 512)
    + one small 1x reduce tail.
  - colmax (-> partial min_n dis): running elementwise TT-max into a
    [128, 8192] fp16 accumulator; the remaining max over the 128
    partitions and over the two n-halves is done on host.
Host does input marshalling (transpose, 2x scale, squared norms, bf16
rounding) and the tiny final gather/means.
"""

import sys

for _p in ("/opt/trn_rl_repo",):
    if _p not in sys.path:
        sys.path.insert(0, _p)

import numpy as np
import ml_dtypes

import concourse.bass as bass
import concourse.mybir as mybir
import concourse.tile as tile
from concourse import bacc
from concourse.bass_utils import run_bass_kernel_spmd

F32 = mybir.dt.float32
F16 = mybir.dt.float16
BF16 = mybir.dt.bfloat16

B, N, C = 4, 8192, 128
NCORES = 8
NH = N // 2              # 4096 rows per core
NT = NH // 128           # 32 n-tile row-blocks per core
SW = 2048                # psum strip width (4 banks)
NS = N // SW             # 4 strips per row-block


def _is_aug(i):
    # ~2/3 of row-blocks use the aug matmul; the rest go via ACT bias +
    # a DVE add. Interleaved (A A N) to smooth engine load; tuned on HW.
    return (i % 3) < 2


_PROGRAM = None


def _build_program():
    nc = bacc.Bacc("TRN2", target_bir_lowering=False, debug=False)

    d_at = nc.dram_tensor("at", [C, NH], BF16, kind="ExternalInput")
    d_bt2 = nc.dram_tensor("bt2", [C, N], BF16, kind="ExternalInput")
    d_augw = nc.dram_tensor("augw", [2, NH], BF16, kind="ExternalInput")
    d_augm = nc.dram_tensor("augm", [2, N], BF16, kind="ExternalInput")
    d_nasq = nc.dram_tensor("nasq", [128, NT], F32, kind="ExternalInput")
    d_nbsq = nc.dram_tensor("nbsq", [1, N], F16, kind="ExternalInput")
    d_rowmax = nc.dram_tensor("rowmax", [128, NT], F32, kind="ExternalOutput")
    d_colacc = nc.dram_tensor("colacc", [128, N], F16, kind="ExternalOutput")

    with tile.TileContext(nc) as tc:
        with (
            tc.tile_pool(name="singles", bufs=1) as singles,
            tc.tile_pool(name="gpool", bufs=3) as gpool,
            tc.tile_pool(name="fpool", bufs=3) as fpool,
            tc.tile_pool(name="hpool", bufs=4) as hpool,
            tc.tile_pool(name="psum", bufs=2, space="PSUM") as psum_pool,
        ):
            at_sb = singles.tile([C, NH], BF16)
            bt2_sb = singles.tile([C, N], BF16)
            augw_sb = singles.tile([2, NH], BF16)
            augm_sb = singles.tile([2, N], BF16)
            nasq_sb = singles.tile([128, NT], F32)
            nbsq_rep = singles.tile([128, N], F16)
            # small tensors first (block 0 needs augw/augm), then the big
            # ones in fine chunks so the first strips can start early
            nc.sync.dma_start(out=augw_sb, in_=d_augw[:])
            nc.sync.dma_start(out=augm_sb, in_=d_augm[:])
            nc.sync.dma_start(out=nasq_sb, in_=d_nasq[:])
            nc.sync.dma_start(out=at_sb[:, 0:512], in_=d_at[:, 0:512])
            for q in range(8):
                nc.sync.dma_start(
                    out=bt2_sb[:, q * (N // 8):(q + 1) * (N // 8)],
                    in_=d_bt2[:, q * (N // 8):(q + 1) * (N // 8)],
                )
            for q in range(1, 8):
                nc.sync.dma_start(
                    out=at_sb[:, q * 512:(q + 1) * 512],
                    in_=d_at[:, q * 512:(q + 1) * 512],
                )
            # broadcast -|f_m'|^2 to all 128 partitions
            nbsq_bcast = bass.AP(
                tensor=d_nbsq[:].tensor,
                offset=d_nbsq[:].offset,
                ap=[[0, 128], [1, N]],
            )
            nc.sync.dma_start(out=nbsq_rep, in_=nbsq_bcast)

            acc = singles.tile([128, N], F16)
            rowmax_sb = singles.tile([128, NT], F32)

            for i in range(NT):
                aug = _is_aug(i)
                w_sl = slice(i * 128, (i + 1) * 128)
                g = gpool.tile([128, N], F16, tag="g")
                for jj in range(NS):
                    ps = psum_pool.tile([128, SW], F32)
                    base = jj * SW
                    for k in range(SW // 512):
                        c_sl = slice(base + k * 512, base + (k + 1) * 512)
                        p_sl = slice(k * 512, (k + 1) * 512)
                        nc.tensor.matmul(
                            out=ps[:, p_sl], lhsT=at_sb[:, w_sl],
                            rhs=bt2_sb[:, c_sl], start=True, stop=not aug,
                        )
                    if aug:
                        for k in range(SW // 512):
                            c_sl = slice(base + k * 512, base + (k + 1) * 512)
                            p_sl = slice(k * 512, (k + 1) * 512)
                            nc.tensor.matmul(
                                out=ps[:, p_sl], lhsT=augw_sb[:, w_sl],
                                rhs=augm_sb[:, c_sl], start=False, stop=True,
                            )
                    gs = g[:, base:base + SW]
                    if aug:
                        nc.scalar.copy(out=gs, in_=ps[:])
                    else:
                        # g = psum + (-|f_n|^2)  (per-partition bias)
                        nc.scalar.activation(
                            out=gs, in_=ps[:],
                            func=mybir.ActivationFunctionType.Identity,
                            bias=nasq_sb[:, i:i + 1], scale=1.0,
                        )
                        # g += -|f_m'|^2  (DVE fp16 2x, in place)
                        nc.vector.tensor_tensor(
                            out=gs, in0=gs,
                            in1=nbsq_rep[:, base:base + SW],
                            op=mybir.AluOpType.add,
                        )
                # colmax running fold (DVE, fp16 2x), one 8192-wide op;
                # first block initializes acc with a 4x-mode copy instead
                if i == 0:
                    nc.vector.tensor_copy(acc[:], g[:])
                else:
                    nc.vector.tensor_tensor(
                        out=acc[:], in0=acc[:], in1=g[:],
                        op=mybir.AluOpType.max,
                    )
                # rowmax fold tree for this row-block (DVE fp16 2x) + 1x tail
                f1 = fpool.tile([128, N // 2], F16, tag="f1")
                nc.vector.tensor_tensor(out=f1[:], in0=g[:, :N // 2], in1=g[:, N // 2:], op=mybir.AluOpType.max)
                f2 = fpool.tile([128, SW], F16, tag="f2")
                nc.vector.tensor_tensor(out=f2[:], in0=f1[:, :SW], in1=f1[:, SW:], op=mybir.AluOpType.max)
                h1 = hpool.tile([128, SW // 2], F16, tag="h1")
                nc.vector.tensor_tensor(out=h1[:], in0=f2[:, :SW // 2], in1=f2[:, SW // 2:], op=mybir.AluOpType.max)
                h2 = hpool.tile([128, SW // 4], F16, tag="h2")
                nc.vector.tensor_tensor(out=h2[:], in0=h1[:, :SW // 4], in1=h1[:, SW // 4:], op=mybir.AluOpType.max)
                nc.vector.tensor_reduce(
                    out=rowmax_sb[:, i:i + 1], in_=h2[:],
                    axis=mybir.AxisListType.X, op=mybir.AluOpType.max,
                )

            nc.sync.dma_start(out=d_rowmax[:], in_=rowmax_sb[:])
            for q in range(2):
                nc.sync.dma_start(
                    out=d_colacc[:, q * (N // 2):(q + 1) * (N // 2)],
                    in_=acc[:, q * (N // 2):(q + 1) * (N // 2)],
                )

    nc.compile()
    return nc


def _get_program():
    global _PROGRAM
    if _PROGRAM is None:
        _PROGRAM = _build_program()
    return _PROGRAM


def _prep_core_inputs(f, f_):
    """Per-core host marshalling: transpose + scale + squared norms."""
    in_maps = []
    for c in range(NCORES):
        b, h = divmod(c, 2)
        A = f[b, h * NH:(h + 1) * NH]        # [4096, 128]
        Bm = f_[b]                           # [8192, 128]
        at = np.ascontiguousarray(A.T.astype(ml_dtypes.bfloat16))
        bt2 = np.ascontiguousarray((2.0 * Bm.T).astype(ml_dtypes.bfloat16))
        asq = (A.astype(np.float64) ** 2).sum(-1).astype(np.float32)
        bsq = (Bm.astype(np.float64) ** 2).sum(-1).astype(np.float32)
        augw = np.ascontiguousarray(np.stack([np.ones(NH, np.float32), -asq]).astype(ml_dtypes.bfloat16))
        augm = np.ascontiguousarray(np.stack([-bsq, np.ones(N, np.float32)]).astype(ml_dtypes.bfloat16))
        # noaug-path constants (full fp32/fp16 precision)
        nasq = np.ascontiguousarray((-asq).reshape(NT, 128).T.astype(np.float32))
        nbsq = np.ascontiguousarray((-bsq).astype(np.float16).reshape(1, N))
        in_maps.append({
            "at": at, "bt2": bt2, "augw": augw, "augm": augm,
            "nasq": nasq, "nbsq": nbsq,
        })
    return in_maps


def _finalize(results):
    """Host-side gather: tiny final reductions + means (fp64)."""
    d_sum = 0.0
    for b in range(B):
        r0 = results[2 * b]
        r1 = results[2 * b + 1]
        f2f_0 = -r0["rowmax"].astype(np.float64).T.reshape(-1)   # [4096]
        f2f_1 = -r1["rowmax"].astype(np.float64).T.reshape(-1)
        mean_f2f = (f2f_0.sum() + f2f_1.sum()) / N
        cm = np.maximum(
            r0["colacc"].astype(np.float32).max(axis=0),
            r1["colacc"].astype(np.float32).max(axis=0),
        ).astype(np.float64)
        mean_f_2f = (-cm).mean()
        d_sum += mean_f2f + mean_f_2f
    return np.float32(d_sum / B)


def kernel(f, f_):
    f = np.asarray(f, dtype=np.float32)
    f_ = np.asarray(f_, dtype=np.float32)
    nc = _get_program()
    in_maps = _prep_core_inputs(f, f_)
    res = run_bass_kernel_spmd(nc, in_maps, list(range(NCORES)))
    return _finalize(res.results)


if __name__ == "__main__":
    rng = np.random.default_rng(0)
    f = rng.standard_normal((B, N, C), dtype=np.float32)
    f_ = rng.standard_normal((B, N, C), dtype=np.float32)
    out = kernel(f, f_)
    print("kernel out:", out)


# revision 14
# speedup vs baseline: 1.3526x; 1.0103x over previous
"""Chamfer distance kernel for Trainium2 (8 NeuronCores, SPMD).

Problem: f, f_ ~ [4, 8192, 128] fp32.
  dis[b,n,m] = ||f[b,n] - f_[b,m]||^2
  out = mean_b( mean_n min_m dis + mean_m min_n dis )

Sharding: 8 cores = (batch b in 0..3) x (n-half h in 0..1).
Core (b,h) computes the [4096, 8192] block of the NEGATED distance matrix
  nd[n,m] = 2<f_n, f_m'> - |f_n|^2 - |f_m'|^2  (= -dis)
on-chip. Matmuls are bf16 (inputs rounded on host), PSUM accumulates fp32.
The two squared-norm terms are applied by a mix of engines chosen to
balance their throughput (PE streams ~0.83 ns/col here):
  - "aug" row-blocks: an extra K=2 matmul accumulates both norm terms
    into PSUM (costs a second PE stream of the strip).
  - "noaug" row-blocks: -|f_n|^2 is applied as a per-partition bias in the
    ACT PSUM->SBUF copy (free), and -|f_m'|^2 by a DVE tensor-add
    (in-place on the fp16 SBUF tile, 2x mode).
Reductions per core (all on fp16 SBUF tiles, DVE 2x mode):
  - rowmax (-> min_m dis for each of 4096 rows): TT-max fold tree over a
    per-block [128, 8192] fp16 tile (8192 -> 4096 -> 2048 -> 1024 -> 512)
    + one small 1x reduce tail.
  - colmax (-> partial min_n dis): running elementwise TT-max into a
    [128, 8192] fp16 accumulator; the remaining max over the 128
    partitions and over the two n-halves is done on host.
Host does input marshalling (transpose, 2x scale, squared norms, bf16
rounding) and the tiny final gather/means.
"""

import sys

for _p in ("/opt/trn_rl_repo",):
    if _p not in sys.path:
        sys.path.insert(0, _p)

import numpy as np
import ml_dtypes

import concourse.bass as bass
import concourse.mybir as mybir
import concourse.tile as tile
from concourse import bacc
from concourse.bass_utils import run_bass_kernel_spmd

F32 = mybir.dt.float32
F16 = mybir.dt.float16
BF16 = mybir.dt.bfloat16

B, N, C = 4, 8192, 128
NCORES = 8
NH = N // 2              # 4096 rows per core
NT = NH // 128           # 32 n-tile row-blocks per core
SW = 2048                # psum strip width (4 banks)
NS = N // SW             # 4 strips per row-block


def _is_aug(i):
    # ~2/3 of row-blocks use the aug matmul; the rest go via ACT bias +
    # a DVE add. Interleaved (A A N) to smooth engine load; tuned on HW.
    return (i % 3) < 2


_PROGRAM = None


def _build_program():
    nc = bacc.Bacc("TRN2", target_bir_lowering=False, debug=False)

    d_at = nc.dram_tensor("at", [C, NH], BF16, kind="ExternalInput")
    d_bt2 = nc.dram_tensor("bt2", [C, N], BF16, kind="ExternalInput")
    d_augw = nc.dram_tensor("augw", [2, NH], BF16, kind="ExternalInput")
    d_augm = nc.dram_tensor("augm", [2, N], BF16, kind="ExternalInput")
    d_nasq = nc.dram_tensor("nasq", [128, NT], F32, kind="ExternalInput")
    d_nbsq = nc.dram_tensor("nbsq", [1, N], F16, kind="ExternalInput")
    d_rowmax = nc.dram_tensor("rowmax", [128, NT], F32, kind="ExternalOutput")
    d_colacc = nc.dram_tensor("colacc", [128, N], F16, kind="ExternalOutput")

    with tile.TileContext(nc) as tc:
        with (
            tc.tile_pool(name="singles", bufs=1) as singles,
            tc.tile_pool(name="gpool", bufs=4) as gpool,
            tc.tile_pool(name="fpool", bufs=3) as fpool,
            tc.tile_pool(name="hpool", bufs=4) as hpool,
            tc.tile_pool(name="psum", bufs=2, space="PSUM") as psum_pool,
        ):
            at_sb = singles.tile([C, NH], BF16)
            bt2_sb = singles.tile([C, N], BF16)
            augw_sb = singles.tile([2, NH], BF16)
            augm_sb = singles.tile([2, N], BF16)
            nasq_sb = singles.tile([128, NT], F32)
            nbsq_rep = singles.tile([128, N], F16)
            # small tensors first (block 0 needs augw/augm), then the big
            # ones in fine chunks so the first strips can start early
            nc.sync.dma_start(out=augw_sb, in_=d_augw[:])
            nc.sync.dma_start(out=augm_sb, in_=d_augm[:])
            nc.sync.dma_start(out=nasq_sb, in_=d_nasq[:])
            nc.sync.dma_start(out=at_sb[:, 0:512], in_=d_at[:, 0:512])
            for q in range(8):
                nc.sync.dma_start(
                    out=bt2_sb[:, q * (N // 8):(q + 1) * (N // 8)],
                    in_=d_bt2[:, q * (N // 8):(q + 1) * (N // 8)],
                )
            for q in range(1, 8):
                nc.sync.dma_start(
                    out=at_sb[:, q * 512:(q + 1) * 512],
                    in_=d_at[:, q * 512:(q + 1) * 512],
                )
            # broadcast -|f_m'|^2 to all 128 partitions
            nbsq_bcast = bass.AP(
                tensor=d_nbsq[:].tensor,
                offset=d_nbsq[:].offset,
                ap=[[0, 128], [1, N]],
            )
            nc.sync.dma_start(out=nbsq_rep, in_=nbsq_bcast)

            acc = singles.tile([128, N], F16)
            rowmax_sb = singles.tile([128, NT], F32)

            for i in range(NT):
                aug = _is_aug(i)
                w_sl = slice(i * 128, (i + 1) * 128)
                g = gpool.tile([128, N], F16, tag="g")
                for jj in range(NS):
                    ps = psum_pool.tile([128, SW], F32)
                    base = jj * SW
                    for k in range(SW // 512):
                        c_sl = slice(base + k * 512, base + (k + 1) * 512)
                        p_sl = slice(k * 512, (k + 1) * 512)
                        nc.tensor.matmul(
                            out=ps[:, p_sl], lhsT=at_sb[:, w_sl],
                            rhs=bt2_sb[:, c_sl], start=True, stop=not aug,
                        )
                    if aug:
                        for k in range(SW // 512):
                            c_sl = slice(base + k * 512, base + (k + 1) * 512)
                            p_sl = slice(k * 512, (k + 1) * 512)
                            nc.tensor.matmul(
                                out=ps[:, p_sl], lhsT=augw_sb[:, w_sl],
                                rhs=augm_sb[:, c_sl], start=False, stop=True,
                            )
                    gs = g[:, base:base + SW]
                    if aug:
                        nc.scalar.copy(out=gs, in_=ps[:])
                    else:
                        # g = psum + (-|f_n|^2)  (per-partition bias)
                        nc.scalar.activation(
                            out=gs, in_=ps[:],
                            func=mybir.ActivationFunctionType.Identity,
                            bias=nasq_sb[:, i:i + 1], scale=1.0,
                        )
                if not aug:
                    # g += -|f_m'|^2  (DVE fp16 2x, one 8192-wide op)
                    nc.vector.tensor_tensor(
                        out=g[:], in0=g[:], in1=nbsq_rep[:],
                        op=mybir.AluOpType.add,
                    )
                # colmax running fold (DVE, fp16 2x), one 8192-wide op;
                # first block initializes acc with a 4x-mode copy instead
                if i == 0:
                    nc.vector.tensor_copy(acc[:], g[:])
                else:
                    nc.vector.tensor_tensor(
                        out=acc[:], in0=acc[:], in1=g[:],
                        op=mybir.AluOpType.max,
                    )
                # rowmax fold tree for this row-block (DVE fp16 2x) + 1x tail
                f1 = fpool.tile([128, N // 2], F16, tag="f1")
                nc.vector.tensor_tensor(out=f1[:], in0=g[:, :N // 2], in1=g[:, N // 2:], op=mybir.AluOpType.max)
                f2 = fpool.tile([128, SW], F16, tag="f2")
                nc.vector.tensor_tensor(out=f2[:], in0=f1[:, :SW], in1=f1[:, SW:], op=mybir.AluOpType.max)
                h1 = hpool.tile([128, SW // 2], F16, tag="h1")
                nc.vector.tensor_tensor(out=h1[:], in0=f2[:, :SW // 2], in1=f2[:, SW // 2:], op=mybir.AluOpType.max)
                h2 = hpool.tile([128, SW // 4], F16, tag="h2")
                nc.vector.tensor_tensor(out=h2[:], in0=h1[:, :SW // 4], in1=h1[:, SW // 4:], op=mybir.AluOpType.max)
                h3 = hpool.tile([128, SW // 8], F16, tag="h3")
                nc.vector.tensor_tensor(out=h3[:], in0=h2[:, :SW // 8], in1=h2[:, SW // 8:], op=mybir.AluOpType.max)
                nc.vector.tensor_reduce(
                    out=rowmax_sb[:, i:i + 1], in_=h3[:],
                    axis=mybir.AxisListType.X, op=mybir.AluOpType.max,
                )

            nc.sync.dma_start(out=d_rowmax[:], in_=rowmax_sb[:])
            for q in range(2):
                nc.sync.dma_start(
                    out=d_colacc[:, q * (N // 2):(q + 1) * (N // 2)],
                    in_=acc[:, q * (N // 2):(q + 1) * (N // 2)],
                )

    nc.compile()
    return nc


def _get_program():
    global _PROGRAM
    if _PROGRAM is None:
        _PROGRAM = _build_program()
    return _PROGRAM


def _prep_core_inputs(f, f_):
    """Per-core host marshalling: transpose + scale + squared norms."""
    in_maps = []
    for c in range(NCORES):
        b, h = divmod(c, 2)
        A = f[b, h * NH:(h + 1) * NH]        # [4096, 128]
        Bm = f_[b]                           # [8192, 128]
        at = np.ascontiguousarray(A.T.astype(ml_dtypes.bfloat16))
        bt2 = np.ascontiguousarray((2.0 * Bm.T).astype(ml_dtypes.bfloat16))
        asq = (A.astype(np.float64) ** 2).sum(-1).astype(np.float32)
        bsq = (Bm.astype(np.float64) ** 2).sum(-1).astype(np.float32)
        augw = np.ascontiguousarray(np.stack([np.ones(NH, np.float32), -asq]).astype(ml_dtypes.bfloat16))
        augm = np.ascontiguousarray(np.stack([-bsq, np.ones(N, np.float32)]).astype(ml_dtypes.bfloat16))
        # noaug-path constants (full fp32/fp16 precision)
        nasq = np.ascontiguousarray((-asq).reshape(NT, 128).T.astype(np.float32))
        nbsq = np.ascontiguousarray((-bsq).astype(np.float16).reshape(1, N))
        in_maps.append({
            "at": at, "bt2": bt2, "augw": augw, "augm": augm,
            "nasq": nasq, "nbsq": nbsq,
        })
    return in_maps


def _finalize(results):
    """Host-side gather: tiny final reductions + means (fp64)."""
    d_sum = 0.0
    for b in range(B):
        r0 = results[2 * b]
        r1 = results[2 * b + 1]
        f2f_0 = -r0["rowmax"].astype(np.float64).T.reshape(-1)   # [4096]
        f2f_1 = -r1["rowmax"].astype(np.float64).T.reshape(-1)
        mean_f2f = (f2f_0.sum() + f2f_1.sum()) / N
        cm = np.maximum(
            r0["colacc"].astype(np.float32).max(axis=0),
            r1["colacc"].astype(np.float32).max(axis=0),
        ).astype(np.float64)
        mean_f_2f = (-cm).mean()
        d_sum += mean_f2f + mean_f_2f
    return np.float32(d_sum / B)


def kernel(f, f_):
    f = np.asarray(f, dtype=np.float32)
    f_ = np.asarray(f_, dtype=np.float32)
    nc = _get_program()
    in_maps = _prep_core_inputs(f, f_)
    res = run_bass_kernel_spmd(nc, in_maps, list(range(NCORES)))
    return _finalize(res.results)


if __name__ == "__main__":
    rng = np.random.default_rng(0)
    f = rng.standard_normal((B, N, C), dtype=np.float32)
    f_ = rng.standard_normal((B, N, C), dtype=np.float32)
    out = kernel(f, f_)
    print("kernel out:", out)


# revision 15
# speedup vs baseline: 1.3665x; 1.0103x over previous
"""Chamfer distance kernel for Trainium2 (8 NeuronCores, SPMD).

Problem: f, f_ ~ [4, 8192, 128] fp32.
  dis[b,n,m] = ||f[b,n] - f_[b,m]||^2
  out = mean_b( mean_n min_m dis + mean_m min_n dis )

Sharding: 8 cores = (batch b in 0..3) x (n-half h in 0..1).
Core (b,h) computes the [4096, 8192] block of the NEGATED distance matrix
  nd[n,m] = 2<f_n, f_m'> - |f_n|^2 - |f_m'|^2  (= -dis)
on-chip. Matmuls are bf16 (inputs rounded on host), PSUM accumulates fp32.
The two squared-norm terms are applied by a mix of engines chosen to
balance their throughput (PE streams ~0.83 ns/col here):
  - "aug" row-blocks: an extra K=2 matmul accumulates both norm terms
    into PSUM (costs a second PE stream of the strip).
  - "noaug" row-blocks: -|f_n|^2 is applied as a per-partition bias in the
    ACT PSUM->SBUF copy (free), and -|f_m'|^2 by a DVE tensor-add
    (in-place on the fp16 SBUF tile, 2x mode).
Reductions per core (all on fp16 SBUF tiles, DVE 2x mode):
  - rowmax (-> min_m dis for each of 4096 rows): TT-max fold tree over a
    per-block [128, 8192] fp16 tile (8192 -> 4096 -> 2048 -> 1024 -> 512)
    + one small 1x reduce tail.
  - colmax (-> partial min_n dis): running elementwise TT-max into a
    [128, 8192] fp16 accumulator; the remaining max over the 128
    partitions and over the two n-halves is done on host.
Host does input marshalling (transpose, 2x scale, squared norms, bf16
rounding) and the tiny final gather/means.
"""

import sys

for _p in ("/opt/trn_rl_repo",):
    if _p not in sys.path:
        sys.path.insert(0, _p)

import numpy as np
import ml_dtypes

import concourse.bass as bass
import concourse.mybir as mybir
import concourse.tile as tile
from concourse import bacc
from concourse.bass_utils import run_bass_kernel_spmd

F32 = mybir.dt.float32
F16 = mybir.dt.float16
BF16 = mybir.dt.bfloat16

B, N, C = 4, 8192, 128
NCORES = 8
NH = N // 2              # 4096 rows per core
NT = NH // 128           # 32 n-tile row-blocks per core
SW = 2048                # psum strip width (4 banks)
NS = N // SW             # 4 strips per row-block


def _is_aug(i):
    # ~2/3 of row-blocks use the aug matmul; the rest go via ACT bias +
    # a DVE add. Interleaved (N A A, noaug first so DVE ramps up sooner);
    # fraction tuned on HW.
    return (i % 3) > 0


_PROGRAM = None


def _build_program():
    nc = bacc.Bacc("TRN2", target_bir_lowering=False, debug=False)

    d_at = nc.dram_tensor("at", [C, NH], BF16, kind="ExternalInput")
    d_bt2 = nc.dram_tensor("bt2", [C, N], BF16, kind="ExternalInput")
    d_augw = nc.dram_tensor("augw", [2, NH], BF16, kind="ExternalInput")
    d_augm = nc.dram_tensor("augm", [2, N], BF16, kind="ExternalInput")
    d_nasq = nc.dram_tensor("nasq", [128, NT], F32, kind="ExternalInput")
    d_nbsq = nc.dram_tensor("nbsq", [1, N], F16, kind="ExternalInput")
    d_rowmax = nc.dram_tensor("rowmax", [128, NT], F32, kind="ExternalOutput")
    d_colacc = nc.dram_tensor("colacc", [128, N], F16, kind="ExternalOutput")

    with tile.TileContext(nc) as tc:
        with (
            tc.tile_pool(name="singles", bufs=1) as singles,
            tc.tile_pool(name="gpool", bufs=4) as gpool,
            tc.tile_pool(name="fpool", bufs=3) as fpool,
            tc.tile_pool(name="hpool", bufs=4) as hpool,
            tc.tile_pool(name="psum", bufs=2, space="PSUM") as psum_pool,
        ):
            at_sb = singles.tile([C, NH], BF16)
            bt2_sb = singles.tile([C, N], BF16)
            augw_sb = singles.tile([2, NH], BF16)
            augm_sb = singles.tile([2, N], BF16)
            nasq_sb = singles.tile([128, NT], F32)
            nbsq_rep = singles.tile([128, N], F16)
            # small tensors first (block 0 needs augw/augm), then the big
            # ones in fine chunks so the first strips can start early
            nc.sync.dma_start(out=augw_sb, in_=d_augw[:])
            nc.sync.dma_start(out=augm_sb, in_=d_augm[:])
            nc.sync.dma_start(out=nasq_sb, in_=d_nasq[:])
            nc.sync.dma_start(out=at_sb[:, 0:512], in_=d_at[:, 0:512])
            for q in range(8):
                nc.sync.dma_start(
                    out=bt2_sb[:, q * (N // 8):(q + 1) * (N // 8)],
                    in_=d_bt2[:, q * (N // 8):(q + 1) * (N // 8)],
                )
            for q in range(1, 8):
                nc.sync.dma_start(
                    out=at_sb[:, q * 512:(q + 1) * 512],
                    in_=d_at[:, q * 512:(q + 1) * 512],
                )
            # broadcast -|f_m'|^2 to all 128 partitions
            nbsq_bcast = bass.AP(
                tensor=d_nbsq[:].tensor,
                offset=d_nbsq[:].offset,
                ap=[[0, 128], [1, N]],
            )
            nc.sync.dma_start(out=nbsq_rep, in_=nbsq_bcast)

            acc = singles.tile([128, N], F16)
            rowmax_sb = singles.tile([128, NT], F32)

            for i in range(NT):
                aug = _is_aug(i)
                w_sl = slice(i * 128, (i + 1) * 128)
                g = gpool.tile([128, N], F16, tag="g")
                for jj in range(NS):
                    ps = psum_pool.tile([128, SW], F32)
                    base = jj * SW
                    for k in range(SW // 512):
                        c_sl = slice(base + k * 512, base + (k + 1) * 512)
                        p_sl = slice(k * 512, (k + 1) * 512)
                        nc.tensor.matmul(
                            out=ps[:, p_sl], lhsT=at_sb[:, w_sl],
                            rhs=bt2_sb[:, c_sl], start=True, stop=not aug,
                        )
                    if aug:
                        for k in range(SW // 512):
                            c_sl = slice(base + k * 512, base + (k + 1) * 512)
                            p_sl = slice(k * 512, (k + 1) * 512)
                            nc.tensor.matmul(
                                out=ps[:, p_sl], lhsT=augw_sb[:, w_sl],
                                rhs=augm_sb[:, c_sl], start=False, stop=True,
                            )
                    gs = g[:, base:base + SW]
                    if aug:
                        nc.scalar.copy(out=gs, in_=ps[:])
                    else:
                        # g = psum + (-|f_n|^2)  (per-partition bias)
                        nc.scalar.activation(
                            out=gs, in_=ps[:],
                            func=mybir.ActivationFunctionType.Identity,
                            bias=nasq_sb[:, i:i + 1], scale=1.0,
                        )
                if not aug:
                    # g += -|f_m'|^2  (DVE fp16 2x, one 8192-wide op)
                    nc.vector.tensor_tensor(
                        out=g[:], in0=g[:], in1=nbsq_rep[:],
                        op=mybir.AluOpType.add,
                    )
                # colmax running fold (DVE, fp16 2x), one 8192-wide op;
                # first block initializes acc with a 4x-mode copy instead
                if i == 0:
                    nc.vector.tensor_copy(acc[:], g[:])
                else:
                    nc.vector.tensor_tensor(
                        out=acc[:], in0=acc[:], in1=g[:],
                        op=mybir.AluOpType.max,
                    )
                # rowmax fold tree for this row-block (DVE fp16 2x) + 1x tail
                f1 = fpool.tile([128, N // 2], F16, tag="f1")
                nc.vector.tensor_tensor(out=f1[:], in0=g[:, :N // 2], in1=g[:, N // 2:], op=mybir.AluOpType.max)
                f2 = fpool.tile([128, SW], F16, tag="f2")
                nc.vector.tensor_tensor(out=f2[:], in0=f1[:, :SW], in1=f1[:, SW:], op=mybir.AluOpType.max)
                h1 = hpool.tile([128, SW // 2], F16, tag="h1")
                nc.vector.tensor_tensor(out=h1[:], in0=f2[:, :SW // 2], in1=f2[:, SW // 2:], op=mybir.AluOpType.max)
                h2 = hpool.tile([128, SW // 4], F16, tag="h2")
                nc.vector.tensor_tensor(out=h2[:], in0=h1[:, :SW // 4], in1=h1[:, SW // 4:], op=mybir.AluOpType.max)
                h3 = hpool.tile([128, SW // 8], F16, tag="h3")
                nc.vector.tensor_tensor(out=h3[:], in0=h2[:, :SW // 8], in1=h2[:, SW // 8:], op=mybir.AluOpType.max)
                nc.vector.tensor_reduce(
                    out=rowmax_sb[:, i:i + 1], in_=h3[:],
                    axis=mybir.AxisListType.X, op=mybir.AluOpType.max,
                )

            nc.sync.dma_start(out=d_rowmax[:], in_=rowmax_sb[:])
            for q in range(2):
                nc.sync.dma_start(
                    out=d_colacc[:, q * (N // 2):(q + 1) * (N // 2)],
                    in_=acc[:, q * (N // 2):(q + 1) * (N // 2)],
                )

    nc.compile()
    return nc


def _get_program():
    global _PROGRAM
    if _PROGRAM is None:
        _PROGRAM = _build_program()
    return _PROGRAM


def _prep_core_inputs(f, f_):
    """Per-core host marshalling: transpose + scale + squared norms."""
    in_maps = []
    for c in range(NCORES):
        b, h = divmod(c, 2)
        A = f[b, h * NH:(h + 1) * NH]        # [4096, 128]
        Bm = f_[b]                           # [8192, 128]
        at = np.ascontiguousarray(A.T.astype(ml_dtypes.bfloat16))
        bt2 = np.ascontiguousarray((2.0 * Bm.T).astype(ml_dtypes.bfloat16))
        asq = (A.astype(np.float64) ** 2).sum(-1).astype(np.float32)
        bsq = (Bm.astype(np.float64) ** 2).sum(-1).astype(np.float32)
        augw = np.ascontiguousarray(np.stack([np.ones(NH, np.float32), -asq]).astype(ml_dtypes.bfloat16))
        augm = np.ascontiguousarray(np.stack([-bsq, np.ones(N, np.float32)]).astype(ml_dtypes.bfloat16))
        # noaug-path constants (full fp32/fp16 precision)
        nasq = np.ascontiguousarray((-asq).reshape(NT, 128).T.astype(np.float32))
        nbsq = np.ascontiguousarray((-bsq).astype(np.float16).reshape(1, N))
        in_maps.append({
            "at": at, "bt2": bt2, "augw": augw, "augm": augm,
            "nasq": nasq, "nbsq": nbsq,
        })
    return in_maps


def _finalize(results):
    """Host-side gather: tiny final reductions + means (fp64)."""
    d_sum = 0.0
    for b in range(B):
        r0 = results[2 * b]
        r1 = results[2 * b + 1]
        f2f_0 = -r0["rowmax"].astype(np.float64).T.reshape(-1)   # [4096]
        f2f_1 = -r1["rowmax"].astype(np.float64).T.reshape(-1)
        mean_f2f = (f2f_0.sum() + f2f_1.sum()) / N
        cm = np.maximum(
            r0["colacc"].astype(np.float32).max(axis=0),
            r1["colacc"].astype(np.float32).max(axis=0),
        ).astype(np.float64)
        mean_f_2f = (-cm).mean()
        d_sum += mean_f2f + mean_f_2f
    return np.float32(d_sum / B)


def kernel(f, f_):
    f = np.asarray(f, dtype=np.float32)
    f_ = np.asarray(f_, dtype=np.float32)
    nc = _get_program()
    in_maps = _prep_core_inputs(f, f_)
    res = run_bass_kernel_spmd(nc, in_maps, list(range(NCORES)))
    return _finalize(res.results)


if __name__ == "__main__":
    rng = np.random.default_rng(0)
    f = rng.standard_normal((B, N, C), dtype=np.float32)
    f_ = rng.standard_normal((B, N, C), dtype=np.float32)
    out = kernel(f, f_)
    print("kernel out:", out)
